# revision 1
# baseline (speedup 1.0000x reference)
"""DiffMHA (differential multi-head attention) block on 8 TRN2 NeuronCores.

Problem: B=4, L=1024, D=1024, H=16 heads (DH=64). Three input streams
(e_v, e_a0, e_a1); Q/K projections per stream, scores summed across
streams, causal-masked softmax, context from the v-stream values,
out-projection + residual + LayerNorm.

Sharding: (batch, head-half) -> 8 cores. Core c handles batch c//2 and
heads (c%2)*8 .. (c%2)*8+8. Each core computes its 8 heads' Q/K/V
projections (512 of 1024 channels), scores + softmax + context, and a
partial out-projection; a pairwise ReduceScatter sums the two partial
out-projections of a batch and splits rows, then each core applies
residual + LayerNorm on its 512 rows.

On-device layout notes:
- All activations feeding matmuls are kept CHANNEL-major ([c, l]
  "transposed" layout) so the PE contracts over partitions with zero
  on-device transposes; the host pre-transposes the embeddings.
- All matmul operands are bf16 (host-converted); PSUM accumulation is
  fp32 and the softmax/LN elementwise pipeline stays fp32.
- softmax runs in scores^T [k, q] layout: the per-q sum over k comes
  free from an extra ones-row appended to V (row 64 of each head's
  ctx PSUM accumulates sum_k attn), so no partition reductions and no
  attn transpose are needed.
- Fold-major schedule: as soon as channel-fold f (2 heads) of Q/K is
  projected for all three streams, those heads' attention runs and the
  fold tiles are recycled.
"""

import os
import sys
import types

import ml_dtypes
import numpy as np

B, L, D, H = 4, 1024, 1024, 16
DH = D // H
HPC = H // 2  # heads per core
C = HPC * DH  # channels per core (512)
SCALE = float(1.0 / np.sqrt(DH))
EPS = 1e-12
NCORES = 8
BF16 = ml_dtypes.bfloat16


def _install_ntff_hook():
    """Recreate antenv.axon_hooks (absent in this image) so
    run_bass_kernel_spmd(trace=True) can capture NTFF profiles."""
    if "antenv.axon_hooks" in sys.modules:
        return
    try:
        from trn_agent_boot.trn_boot import _ntff_profile_via_ctypes

        hook = _ntff_profile_via_ctypes("/opt/axon/libaxon_pjrt.so")
    except Exception:
        hook = None
    mod = types.ModuleType("antenv.axon_hooks")
    mod.get_axon_ntff_profile_hook = lambda: hook
    mod.set_axon_ntff_profile_hook = lambda h: None
    sys.modules["antenv.axon_hooks"] = mod


_install_ntff_hook()

import concourse.bass as bass  # noqa: E402
import concourse.mybir as mybir  # noqa: E402
import concourse.tile as tile  # noqa: E402
from concourse import bacc  # noqa: E402
from concourse.bass_utils import run_bass_kernel_spmd  # noqa: E402

F32 = mybir.dt.float32
BF = mybir.dt.bfloat16
AF = mybir.ActivationFunctionType
ALU = mybir.AluOpType

_NC_CACHE = {}
LAST_RESULT = None

NQF = C // 128  # 4 channel folds per stream (2 heads each)
NLT = L // 128  # 8 l-tiles
NDT = D // 128  # 8 d-tiles (contraction)
NKT = L // 128  # 8 k-tiles
NRF = (L // 2) // 128  # 4 row folds for LN
STREAMS = ("v", "a0", "a1")


def build_nc():
    nc = bacc.Bacc("TRN2", target_bir_lowering=False, debug=False, num_devices=NCORES)

    # ---- DRAM parameters (per-core shards, host-prepped) ----
    xt = {s: nc.declare_dram_parameter(f"xt_{s}", [D, L], BF, isOutput=False) for s in STREAMS}
    # W fold-sliced on host: [NQF, D, 128]
    wq = {s: nc.declare_dram_parameter(f"wq_{s}", [NQF, D, 128], BF, isOutput=False) for s in STREAMS}
    wk = {s: nc.declare_dram_parameter(f"wk_{s}", [NQF, D, 128], BF, isOutput=False) for s in STREAMS}
    wv = nc.declare_dram_parameter("wv", [D, C], BF, isOutput=False)
    wout = nc.declare_dram_parameter("wout", [C, D], BF, isOutput=False)
    bq = {s: nc.declare_dram_parameter(f"bq_{s}", [C], F32, isOutput=False) for s in STREAMS}
    bk = {s: nc.declare_dram_parameter(f"bk_{s}", [C], F32, isOutput=False) for s in STREAMS}
    bv = nc.declare_dram_parameter("bv", [1, C], BF, isOutput=False)
    bout_half = nc.declare_dram_parameter("bout_half", [1, D], BF, isOutput=False)
    maskt = nc.declare_dram_parameter("maskt", [L, L], BF, isOutput=False)
    ev_res = nc.declare_dram_parameter("ev_res", [L // 2, D], F32, isOutput=False)
    gamma = nc.declare_dram_parameter("gamma", [1, D], F32, isOutput=False)
    beta = nc.declare_dram_parameter("beta", [1, D], F32, isOutput=False)
    out = nc.declare_dram_parameter("out", [L // 2, D], F32, isOutput=True)

    with tile.TileContext(nc) as tc:
        with (
            tc.tile_pool(name="persist", bufs=1) as persist,
            tc.tile_pool(name="xtp", bufs=1) as xtp,
            tc.tile_pool(name="wf", bufs=8) as wf,
            tc.tile_pool(name="qkf", bufs=2) as qkf,
            tc.tile_pool(name="small", bufs=4) as small,
            tc.tile_pool(name="attn", bufs=4) as attn_pool,
            tc.tile_pool(name="ln", bufs=2) as ln_pool,
            tc.tile_pool(name="proj_ps", bufs=2, space="PSUM") as proj_ps,
            tc.tile_pool(name="sc_ps", bufs=2, space="PSUM") as sc_ps,
            tc.tile_pool(name="ctx_ps", bufs=2, space="PSUM") as ctx_ps,
            tc.tile_pool(name="dram", bufs=1, space="DRAM") as dram,
        ):
            # ---- persistent SBUF tensors ----
            vnat = persist.tile([128, NLT, HPC, DH + 1], BF, tag="vnat")
            ctxt = persist.tile([128, NQF, L], BF, tag="ctxt")
            maskt_sb = persist.tile([128, NKT, L], BF, tag="maskt")
            ones_b = persist.tile([1, L], BF, tag="ones")
            gb_bc = persist.tile([128, 2, D], F32, tag="gbbc")
            bout_sb = persist.tile([1, D], BF, tag="boutsb")
            bv_sb = persist.tile([1, C], BF, tag="bvsb")
            wout_sb = persist.tile([128, NQF, D], BF, tag="woutsb")
            eps_sb = persist.tile([128, 1], F32, tag="eps")
            bq_sb = {
                s: persist.tile([128, NQF], F32, tag=f"bq{s}", name=f"bq_sb_{s}")
                for s in STREAMS
            }
            bk_sb = {
                s: persist.tile([128, NQF], F32, tag=f"bk{s}", name=f"bk_sb_{s}")
                for s in STREAMS
            }

            nc.vector.memset(ones_b[:, :], 1.0)
            nc.vector.memset(eps_sb[:, :], EPS)
            # V ones-column (feeds the softmax-sum rows)
            nc.vector.memset(vnat[:, :, :, DH : DH + 1], 1.0)

            nc.sync.dma_start(
                out=maskt_sb[:, :, :],
                in_=maskt[:, :].rearrange("(kt p) q -> p kt q", p=128),
            )
            nc.sync.dma_start(out=bout_sb[:, :], in_=bout_half[:, :])
            nc.sync.dma_start(out=bv_sb[:, :], in_=bv[:, :])
            nc.sync.dma_start(
                out=wout_sb[:, :, :],
                in_=wout[:, :].rearrange("(cf p) d -> p cf d", p=128),
            )
            for s in STREAMS:
                nc.sync.dma_start(
                    out=bq_sb[s][:, :], in_=bq[s][:].rearrange("(f p) -> p f", p=128)
                )
                nc.sync.dma_start(
                    out=bk_sb[s][:, :], in_=bk[s][:].rearrange("(f p) -> p f", p=128)
                )
            gsb = small.tile([1, D], F32, tag="gsb", bufs=1)
            bsb = small.tile([1, D], F32, tag="bsb", bufs=1)
            nc.sync.dma_start(out=gsb[:, :], in_=gamma[:, :])
            nc.sync.dma_start(out=bsb[:, :], in_=beta[:, :])
            nc.gpsimd.partition_broadcast(gb_bc[:, 0, :], gsb[:, :])
            nc.gpsimd.partition_broadcast(gb_bc[:, 1, :], bsb[:, :])

            # ---- embeddings (channel-major), resident ----
            xt_sb = {}
            for s in STREAMS:
                t = xtp.tile([128, NDT, L], BF, tag=f"xt{s}", name=f"xt_sb_{s}")
                nc.sync.dma_start(
                    out=t[:, :, :],
                    in_=xt[s][:, :].rearrange("(dt p) l -> p dt l", p=128),
                )
                xt_sb[s] = t

            # ---- V projection (natural [l, c] layout + ones column) ----
            wv_sb = persist.tile([128, NDT, C], BF, tag="wvsb")
            nc.sync.dma_start(
                out=wv_sb[:, :, :],
                in_=wv[:, :].rearrange("(dt p) c -> p dt c", p=128),
            )
            for lf in range(NLT):
                ps = proj_ps.tile([128, C], F32, tag="proj")
                for dt in range(NDT):
                    nc.tensor.matmul(
                        ps[:, :],
                        xt_sb["v"][:, dt, lf * 128 : (lf + 1) * 128],
                        wv_sb[:, dt, :],
                        start=(dt == 0),
                        stop=False,
                    )
                # + bias via ones-row rank-1 update
                nc.tensor.matmul(
                    ps[:, :],
                    ones_b[:, lf * 128 : (lf + 1) * 128],
                    bv_sb[:, :],
                    start=False,
                    stop=True,
                )
                nc.scalar.copy(vnat[:, lf, :, 0:DH], ps[:, :])

            # ---- fold-major: project fold f for all streams, then run
            #      attention for heads 2f and 2f+1 ----
            for f in range(NQF):
                qtf, ktf = {}, {}
                for s in STREAMS:
                    wq_t = wf.tile([128, NDT, 128], BF, tag="w", name=f"wq_{s}{f}")
                    wk_t = wf.tile([128, NDT, 128], BF, tag="w", name=f"wk_{s}{f}")
                    nc.sync.dma_start(
                        out=wq_t[:, :, :],
                        in_=wq[s][f, :, :].rearrange("(dt p) c -> p dt c", p=128),
                    )
                    nc.sync.dma_start(
                        out=wk_t[:, :, :],
                        in_=wk[s][f, :, :].rearrange("(dt p) c -> p dt c", p=128),
                    )
                    for which, w_t, b_t, store in (
                        ("q", wq_t, bq_sb[s], qtf),
                        ("k", wk_t, bk_sb[s], ktf),
                    ):
                        ps = proj_ps.tile([128, L], F32, tag="proj")
                        for dt in range(NDT):
                            for lh in range(2):
                                nc.tensor.matmul(
                                    ps[:, lh * 512 : (lh + 1) * 512],
                                    w_t[:, dt, :],
                                    xt_sb[s][:, dt, lh * 512 : (lh + 1) * 512],
                                    start=(dt == 0),
                                    stop=(dt == NDT - 1),
                                )
                        dst = qkf.tile(
                            [128, L], BF, tag=f"{which}t{s}", name=f"{which}t_{s}{f}"
                        )
                        nc.scalar.activation(
                            dst[:, :], ps[:, :], AF.Identity, bias=b_t[:, f : f + 1]
                        )
                        store[s] = dst

                for hh in range(2):  # head within fold
                    h = 2 * f + hh
                    p0 = hh * 64
                    for qh in range(2):
                        qsl = slice(qh * 512, (qh + 1) * 512)
                        cps = ctx_ps.tile([DH + 1, 512], F32, tag="ctx")
                        for kt_i in range(NKT):
                            ksl = slice(kt_i * 128, (kt_i + 1) * 128)
                            sps = sc_ps.tile([128, 512], F32, tag="sc")
                            for i, s in enumerate(STREAMS):
                                nc.tensor.matmul(
                                    sps[:, :],
                                    ktf[s][p0 : p0 + 64, ksl],
                                    qtf[s][p0 : p0 + 64, qsl],
                                    start=(i == 0),
                                    stop=(i == 2),
                                )
                            # add (mask / SCALE), then exp(SCALE * x)
                            nc.vector.tensor_add(
                                sps[:, :], sps[:, :], maskt_sb[:, kt_i, qsl]
                            )
                            attn_sb = attn_pool.tile([128, 512], BF, tag="attn")
                            nc.scalar.activation(
                                attn_sb[:, :], sps[:, :], AF.Exp, scale=SCALE
                            )
                            nc.tensor.matmul(
                                cps[:, :],
                                vnat[:, kt_i, h, :],
                                attn_sb[:, :],
                                start=(kt_i == 0),
                                stop=(kt_i == NKT - 1),
                            )
                        inv = small.tile([1, 512], F32, tag="inv", bufs=2)
                        nc.vector.reciprocal(inv[:, :], cps[DH : DH + 1, :])
                        inv_bc = small.tile([64, 512], F32, tag="invbc", bufs=2)
                        nc.gpsimd.partition_broadcast(inv_bc[:, :], inv[:, :])
                        nc.vector.tensor_mul(
                            ctxt[p0 : p0 + 64, f, qsl], cps[0:DH, :], inv_bc[:, :]
                        )

            # ---- partial out-projection ----
            partial = dram.tile([L, D], F32)
            for lt in range(NLT):
                lsl = slice(lt * 128, (lt + 1) * 128)
                ops = proj_ps.tile([128, D], F32, tag="proj")
                for dh_i in range(2):
                    dsl = slice(dh_i * 512, (dh_i + 1) * 512)
                    for cf in range(NQF):
                        nc.tensor.matmul(
                            ops[:, dsl],
                            ctxt[:, cf, lsl],
                            wout_sb[:, cf, dsl],
                            start=(cf == 0),
                            stop=False,
                        )
                    nc.tensor.matmul(
                        ops[:, dsl],
                        ones_b[:, lsl],
                        bout_sb[:, dsl],
                        start=False,
                        stop=True,
                    )
                op_sb = ln_pool.tile([128, D], F32, tag="x")
                nc.scalar.copy(op_sb[:, :], ops[:, :])
                nc.sync.dma_start(out=partial[lsl, :], in_=op_sb[:, :])

            # ---- ReduceScatter over batch pairs ----
            rs_out = dram.tile([L // 2, D], F32)
            nc.gpsimd.collective_compute(
                "ReduceScatter",
                ALU.add,
                replica_groups=[[0, 1], [2, 3], [4, 5], [6, 7]],
                ins=[partial.opt()],
                outs=[rs_out.opt()],
            )

            # ---- residual + LayerNorm on own 512 rows ----
            for rf in range(NRF):
                rsl = slice(rf * 128, (rf + 1) * 128)
                x_sb = ln_pool.tile([128, D], F32, tag="x")
                nc.sync.dma_start(out=x_sb[:, :], in_=rs_out[rsl, :])
                ev_sb = ln_pool.tile([128, D], F32, tag="ev")
                nc.sync.dma_start(out=ev_sb[:, :], in_=ev_res[rsl, :])
                nc.vector.tensor_add(x_sb[:, :], x_sb[:, :], ev_sb[:, :])
                stats = small.tile([128, 2, 6], F32, tag="stats")
                nc.vector.bn_stats(out=stats[:, 0, :], in_=x_sb[:, 0:512])
                nc.vector.bn_stats(out=stats[:, 1, :], in_=x_sb[:, 512:1024])
                mv = small.tile([128, 2], F32, tag="mv")
                nc.vector.bn_aggr(out=mv[:, :], in_=stats[:, :, :])
                std = small.tile([128, 1], F32, tag="std")
                nc.scalar.activation(std[:, :], mv[:, 1:2], AF.Sqrt, bias=eps_sb[:, :])
                rstd = small.tile([128, 1], F32, tag="rstd")
                nc.vector.reciprocal(rstd[:, :], std[:, :])
                negmb = small.tile([128, 1], F32, tag="negmb")
                nc.vector.scalar_tensor_tensor(
                    negmb[:, :],
                    mv[:, 0:1],
                    -1.0,
                    rstd[:, :],
                    op0=ALU.mult,
                    op1=ALU.mult,
                )
                nc.scalar.activation(
                    x_sb[:, :],
                    x_sb[:, :],
                    AF.Identity,
                    bias=negmb[:, :],
                    scale=rstd[:, :],
                )
                nc.vector.tensor_mul(x_sb[:, :], x_sb[:, :], gb_bc[:, 0, :])
                nc.vector.tensor_add(x_sb[:, :], x_sb[:, :], gb_bc[:, 1, :])
                nc.sync.dma_start(out=out[rsl, :], in_=x_sb[:, :])

    nc.compile()
    return nc


def _get_nc():
    if "nc" not in _NC_CACHE:
        _NC_CACHE["nc"] = build_nc()
    return _NC_CACHE["nc"]


def kernel(
    e_v, e_a0, e_a1, Wqv, bqv, Wkv, bkv, Wvv, bvv,
    Wqa0, bqa0, Wka0, bka0, Wqa1, bqa1, Wka1, bka1,
    Wout, bout, ln_gamma, ln_beta, attn_mask,
):
    global LAST_RESULT
    f = np.asarray
    e_v, e_a0, e_a1 = f(e_v), f(e_a0), f(e_a1)
    attn_mask = f(attn_mask)
    c32 = lambda a: np.ascontiguousarray(a, dtype=np.float32)
    cbf = lambda a: np.ascontiguousarray(np.asarray(a, dtype=np.float32).astype(BF16))

    wq_full = {"v": f(Wqv), "a0": f(Wqa0), "a1": f(Wqa1)}
    wk_full = {"v": f(Wkv), "a0": f(Wka0), "a1": f(Wka1)}
    bq_full = {"v": f(bqv), "a0": f(bqa0), "a1": f(bqa1)}
    bk_full = {"v": f(bkv), "a0": f(bka0), "a1": f(bka1)}

    xts = {}
    maskts = {}
    for b in range(B):
        xts[b] = {
            "v": cbf(e_v[b].T),
            "a0": cbf(e_a0[b].T),
            "a1": cbf(e_a1[b].T),
        }
        maskts[b] = cbf(attn_mask[b, 0].T * (1.0 / SCALE))

    def fold_slice(w, S):
        # [D, C] slice -> [NQF, D, 128] fold-major
        ws = np.asarray(w[:, S], dtype=np.float32)  # [D, C]
        return np.ascontiguousarray(
            ws.reshape(D, NQF, 128).transpose(1, 0, 2).astype(BF16)
        )

    in_maps = []
    for c in range(NCORES):
        b, hh = c // 2, c % 2
        S = slice(hh * C, (hh + 1) * C)
        m = {}
        for s in STREAMS:
            m[f"xt_{s}"] = xts[b][s]
            m[f"wq_{s}"] = fold_slice(wq_full[s], S)
            m[f"wk_{s}"] = fold_slice(wk_full[s], S)
            m[f"bq_{s}"] = c32(bq_full[s][S])
            m[f"bk_{s}"] = c32(bk_full[s][S])
        m["wv"] = cbf(f(Wvv)[:, S])
        m["bv"] = cbf(f(bvv)[S]).reshape(1, C)
        m["wout"] = cbf(f(Wout)[S, :])
        m["bout_half"] = cbf(f(bout) * 0.5).reshape(1, D)
        m["maskt"] = maskts[b]
        m["ev_res"] = c32(e_v[b, hh * 512 : (hh + 1) * 512, :])
        m["gamma"] = c32(f(ln_gamma)).reshape(1, D)
        m["beta"] = c32(f(ln_beta)).reshape(1, D)
        in_maps.append(m)

    nc = _get_nc()
    trace = bool(os.environ.get("KERNEL_TRACE"))
    res = run_bass_kernel_spmd(
        nc, in_maps, core_ids=list(range(NCORES)), trace=trace
    )
    LAST_RESULT = res

    out_full = np.empty((B, L, D), dtype=np.float32)
    for c in range(NCORES):
        b, hh = c // 2, c % 2
        out_full[b, hh * 512 : (hh + 1) * 512, :] = res.results[c]["out"]
    return out_full



# revision 11
# speedup vs baseline: 1.6033x; 1.6033x over previous
"""DiffMHA (differential multi-head attention) block on 8 TRN2 NeuronCores.

Problem: B=4, L=1024, D=1024, H=16 heads (DH=64). Three input streams
(e_v, e_a0, e_a1); Q/K projections per stream, scores summed across
streams, causal-masked softmax, context from the v-stream values,
out-projection + residual + LayerNorm.

Sharding: (batch, head-half) -> 8 cores. Core c handles batch c//2 and
heads (c%2)*8 .. (c%2)*8+8. Each core computes its 8 heads' Q/K/V
projections (512 of 1024 channels), scores + softmax + context, and a
partial out-projection; four pairwise bf16 ReduceScatters (one per
256-row group) sum the two partial out-projections of a batch and
split rows; each core then applies residual + LayerNorm on its four
128-row stripes.

Performance notes (v3):
- PE p-state: the tensor engine only reaches 2.4 GHz after ~3 us of
  continuous execution; a warmup matmul burst starts the ramp while
  the first DMAs land, and the schedule keeps the PE queue dense
  (proj of fold f+2 emitted between attention folds).
- DMA staggering: only the V-projection critical path (wv, xt_v,
  wq_v, wk_v) is triggered at t=0 from the sync engine; every other
  load is triggered from the scalar engine interleaved between
  PSUM->SBUF copies, so the critical 3 MB is not bandwidth-starved.
- A zero-byte-ish dummy ReduceScatter at t=0 pre-pays the ~20 us
  collective channel setup so the first real RS is fast.
- Causal skip: q-half 0 computes only k-tiles 0..3; mask adds only on
  the 8 diagonal blocks per head; mask payload is one 512x512 block.
- Q/K biases ride the copy instructions (scalar activation for Q,
  vector tensor_scalar for K - splitting engines halves the copy
  backlog in front of the exps). V/out biases fold into the host-side
  residual (bvv @ Wout + bout), exact for all inputs.
- softmax in scores^T [k, q] layout: per-q sums via a ones-column in
  V; normalize = denom copy to SBUF + reciprocal_approx_fast +
  in-place partition broadcast (vector.reciprocal's 2.7us multipass
  and PSUM-input approx-reciprocal are both avoided).
- Epilogue: out-proj for rows 0..511 interleaves between the last
  fold's q-halves; PSUM->SBUF copies on gpsimd; 4 bf16 ReduceScatters
  overlap attention; LayerNorm skips gamma/beta when they are
  ones/zeros (runtime-selected graph variant).
"""

import os
import sys
import types

import ml_dtypes
import numpy as np

B, L, D, H = 4, 1024, 1024, 16
DH = D // H
HPC = H // 2  # heads per core
C = HPC * DH  # channels per core (512)
SCALE = float(1.0 / np.sqrt(DH))
EPS = 1e-12
NCORES = 8
BF16 = ml_dtypes.bfloat16


def _install_ntff_hook():
    """Recreate antenv.axon_hooks (absent in this image) so
    run_bass_kernel_spmd(trace=True) can capture NTFF profiles."""
    if "antenv.axon_hooks" in sys.modules:
        return
    try:
        from trn_agent_boot.trn_boot import _ntff_profile_via_ctypes

        hook = _ntff_profile_via_ctypes("/opt/axon/libaxon_pjrt.so")
    except Exception:
        hook = None
    mod = types.ModuleType("antenv.axon_hooks")
    mod.get_axon_ntff_profile_hook = lambda: hook
    mod.set_axon_ntff_profile_hook = lambda h: None
    sys.modules["antenv.axon_hooks"] = mod


_install_ntff_hook()

import concourse.bass as bass  # noqa: E402
import concourse.mybir as mybir  # noqa: E402
import concourse.tile as tile  # noqa: E402
from concourse import bacc  # noqa: E402
from concourse.bass_utils import run_bass_kernel_spmd  # noqa: E402

F32 = mybir.dt.float32
BF = mybir.dt.bfloat16
AF = mybir.ActivationFunctionType
ALU = mybir.AluOpType

_NC_CACHE = {}
LAST_RESULT = None

NQF = C // 128  # 4 channel folds per stream (2 heads each)
NLT = L // 128  # 8 l-tiles
NDT = D // 128  # 8 d-tiles (contraction)
NKT = L // 128  # 8 k-tiles
STREAMS = ("v", "a0", "a1")
PAIRS = [[0, 1], [2, 3], [4, 5], [6, 7]]
N_WARM = 30


def build_nc(apply_gb: bool):
    nc = bacc.Bacc("TRN2", target_bir_lowering=False, debug=False, num_devices=NCORES)

    # ---- DRAM parameters (per-core shards, host-packed to SBUF layout) ----
    xt = {s: nc.declare_dram_parameter(f"xt_{s}", [128, NDT * L], BF, isOutput=False) for s in STREAMS}
    wq = {s: nc.declare_dram_parameter(f"wq_{s}", [128, NQF * NDT * 128], BF, isOutput=False) for s in STREAMS}
    wk = {s: nc.declare_dram_parameter(f"wk_{s}", [128, NQF * NDT * 128], BF, isOutput=False) for s in STREAMS}
    wv = nc.declare_dram_parameter("wv", [128, NDT * C], BF, isOutput=False)
    wout = nc.declare_dram_parameter("wout", [128, NQF * D], BF, isOutput=False)
    bq = {s: nc.declare_dram_parameter(f"bq_{s}", [128, NQF], F32, isOutput=False) for s in STREAMS}
    bk = {s: nc.declare_dram_parameter(f"bk_{s}", [128, NQF], F32, isOutput=False) for s in STREAMS}
    maskd = nc.declare_dram_parameter("maskd", [128, 4 * 512], BF, isOutput=False)
    ev_res = nc.declare_dram_parameter("ev_res", [128, 4 * D], BF, isOutput=False)
    gamma = nc.declare_dram_parameter("gamma", [1, D], BF, isOutput=False)
    beta = nc.declare_dram_parameter("beta", [1, D], BF, isOutput=False)
    out = nc.declare_dram_parameter("out", [C, D], F32, isOutput=True)

    with tile.TileContext(nc) as tc:
        with (
            tc.tile_pool(name="persist", bufs=1) as persist,
            tc.tile_pool(name="qka", bufs=2) as qka,
            tc.tile_pool(name="small", bufs=4) as small,
            tc.tile_pool(name="attn", bufs=3) as attn_pool,
            tc.tile_pool(name="ln", bufs=2) as ln_pool,
            tc.tile_pool(name="ob", bufs=2) as ob_pool,
            tc.tile_pool(name="qk_ps", bufs=2, space="PSUM") as qk_ps,
            tc.tile_pool(name="sc_ps", bufs=3, space="PSUM") as sc_ps,
            tc.tile_pool(name="ctx_ps", bufs=3, space="PSUM") as ctx_ps,
            tc.tile_pool(name="dram", bufs=1, space="DRAM") as dram,
        ):
            # ---- persistent SBUF tensors ----
            warm = persist.tile([128, 512], BF, tag="warm")
            vnat = persist.tile([128, NLT, HPC, DH + 1], BF, tag="vnat")
            ctxt = persist.tile([128, NQF, L], BF, tag="ctxt")
            maskd_sb = persist.tile([128, 4, 512], BF, tag="maskd")
            wv_sb = persist.tile([128, NDT, C], BF, tag="wvsb")
            wout_sb = persist.tile([128, NQF, D], BF, tag="woutsb")
            ev_sb = persist.tile([128, 4, D], BF, tag="evsb")
            eps_sb = persist.tile([128, 1], F32, tag="eps")
            if apply_gb:
                gb_bc = persist.tile([128, 2, D], BF, tag="gbbc")
            xt_sb = {
                s: persist.tile([128, NDT, L], BF, tag=f"xt{s}", name=f"xt_sb_{s}")
                for s in STREAMS
            }
            wq_sb = {
                s: persist.tile([128, NQF, NDT, 128], BF, tag=f"wq{s}", name=f"wq_sb_{s}")
                for s in STREAMS
            }
            wk_sb = {
                s: persist.tile([128, NQF, NDT, 128], BF, tag=f"wk{s}", name=f"wk_sb_{s}")
                for s in STREAMS
            }
            bq_sb = {
                s: persist.tile([128, NQF], F32, tag=f"bq{s}", name=f"bq_sb_{s}")
                for s in STREAMS
            }
            bk_sb = {
                s: persist.tile([128, NQF], F32, tag=f"bk{s}", name=f"bk_sb_{s}")
                for s in STREAMS
            }
            # v-stream q/k tiles persist for all folds; a0/a1 ride a ring
            qt_v = [persist.tile([128, L], BF, tag=f"qtv{f}", name=f"qt_v{f}") for f in range(NQF)]
            kt_v = [persist.tile([128, L], BF, tag=f"ktv{f}", name=f"kt_v{f}") for f in range(NQF)]

            # DRAM scratch for the pairwise reduce-scatters
            partial_g = [dram.tile([256, D], BF, tag=f"part{g}", name=f"partial{g}") for g in range(4)]
            rs_g = [dram.tile([128, D], BF, tag=f"rs{g}", name=f"rsout{g}") for g in range(4)]
            wcc_in = dram.tile([2, 64], BF, tag="wccin")
            wcc_out = dram.tile([1, 64], BF, tag="wccout")

            # ---- t=0: critical-path loads only (sync engine) ----
            nc.sync.dma_start(out=wv_sb[:, :, :], in_=wv[:, :].rearrange("p (dt c) -> p dt c", c=C))
            nc.sync.dma_start(out=xt_sb["v"][:, :, :], in_=xt["v"][:, :].rearrange("p (dt l) -> p dt l", l=L))
            nc.sync.dma_start(out=wq_sb["v"][:, :, :, :], in_=wq["v"][:, :].rearrange("p (f dt c) -> p f dt c", f=NQF, dt=NDT))
            nc.sync.dma_start(out=wk_sb["v"][:, :, :, :], in_=wk["v"][:, :].rearrange("p (f dt c) -> p f dt c", f=NQF, dt=NDT))
            # dummy collective: pre-pays the CC channel setup (~20 us)
            nc.gpsimd.collective_compute(
                "ReduceScatter", ALU.add, replica_groups=PAIRS,
                ins=[wcc_in.opt()], outs=[wcc_out.opt()],
            )
            nc.vector.memset(warm[:, :], 1.0)
            nc.vector.memset(vnat[:, :, :, DH : DH + 1], 1.0)
            nc.vector.memset(eps_sb[:, :], EPS)

            # ---- deferred loads: triggered from the scalar engine between
            #      copies so they start only after the critical path is in ----
            deferred = []

            def defer(fn):
                deferred.append(fn)

            def drain(n=2):
                for _ in range(n):
                    if deferred:
                        deferred.pop(0)()

            for s in STREAMS:
                defer(lambda s=s: nc.scalar.dma_start(out=bq_sb[s][:, :], in_=bq[s][:, :]))
                defer(lambda s=s: nc.scalar.dma_start(out=bk_sb[s][:, :], in_=bk[s][:, :]))
            for s in ("a0", "a1"):
                defer(lambda s=s: nc.scalar.dma_start(
                    out=xt_sb[s][:, :, :], in_=xt[s][:, :].rearrange("p (dt l) -> p dt l", l=L)))

            def defer_fold_w(s, f):
                defer(lambda s=s, f=f: nc.scalar.dma_start(
                    out=wq_sb[s][:, f, :, :],
                    in_=wq[s][:, f * 1024 : (f + 1) * 1024].rearrange("p (dt c) -> p dt c", c=128)))
                defer(lambda s=s, f=f: nc.scalar.dma_start(
                    out=wk_sb[s][:, f, :, :],
                    in_=wk[s][:, f * 1024 : (f + 1) * 1024].rearrange("p (dt c) -> p dt c", c=128)))

            for s in ("a0", "a1"):
                defer_fold_w(s, 0)
            defer(lambda: nc.scalar.dma_start(
                out=maskd_sb[:, :, :], in_=maskd[:, :].rearrange("p (i q) -> p i q", q=512)))
            for s in ("a0", "a1"):
                defer_fold_w(s, 1)
            for f in (2, 3):
                for s in ("a0", "a1"):
                    defer_fold_w(s, f)
            defer(lambda: nc.scalar.dma_start(
                out=wout_sb[:, :, :], in_=wout[:, :].rearrange("p (cf d) -> p cf d", d=D)))
            defer(lambda: nc.scalar.dma_start(
                out=ev_sb[:, :, :], in_=ev_res[:, :].rearrange("p (i d) -> p i d", d=D)))
            if apply_gb:
                gsb = small.tile([1, D], BF, tag="gsb", bufs=1)
                bsb = small.tile([1, D], BF, tag="bsb", bufs=1)
                defer(lambda: nc.scalar.dma_start(out=gsb[:, :], in_=gamma[:, :]))
                defer(lambda: nc.scalar.dma_start(out=bsb[:, :], in_=beta[:, :]))

            # ---- PE warmup: ramp the p-state while DMAs land ----
            for i in range(N_WARM):
                wps = sc_ps.tile([128, 512], F32, tag="sc", name=f"warmps{i}")
                nc.tensor.matmul(wps[:, :], warm[:, 0:128], warm[:, :], start=True, stop=True)

            # ---- V projection (natural [l, c] layout + ones column) ----
            for lf in range(NLT):
                ps = qk_ps.tile([128, C], F32, tag="qk", name=f"vps{lf}")
                for dt in range(NDT):
                    nc.tensor.matmul(
                        ps[:, :],
                        xt_sb["v"][:, dt, lf * 128 : (lf + 1) * 128],
                        wv_sb[:, dt, :],
                        start=(dt == 0),
                        stop=(dt == NDT - 1),
                    )
                nc.scalar.copy(vnat[:, lf, :, 0:DH], ps[:, :])
                drain(2)

            # ---- Q/K projections: Q copy on scalar, K copy on vector ----
            def emit_qkproj(s, f, qdst, kdst):
                for which, w_sb, b_sb, dst in (
                    ("q", wq_sb[s], bq_sb[s], qdst),
                    ("k", wk_sb[s], bk_sb[s], kdst),
                ):
                    for lh in range(2):
                        ps = qk_ps.tile([128, 512], F32, tag="qk", name=f"pj{s}{f}{lh}")
                        for dt in range(NDT):
                            nc.tensor.matmul(
                                ps[:, :],
                                w_sb[:, f, dt, :],
                                xt_sb[s][:, dt, lh * 512 : (lh + 1) * 512],
                                start=(dt == 0),
                                stop=(dt == NDT - 1),
                            )
                        dsl = dst[:, lh * 512 : (lh + 1) * 512]
                        if which == "q":
                            nc.scalar.activation(dsl, ps[:, :], AF.Identity, bias=b_sb[:, f : f + 1])
                            drain(2)
                        else:
                            nc.vector.tensor_scalar(
                                dsl, ps[:, :], b_sb[:, f : f + 1], None, ALU.add
                            )

            def qka_tiles(f):
                q0 = qka.tile([128, L], BF, tag="qa0", name=f"qt_a0_{f}")
                k0 = qka.tile([128, L], BF, tag="ka0", name=f"kt_a0_{f}")
                q1 = qka.tile([128, L], BF, tag="qa1", name=f"qt_a1_{f}")
                k1 = qka.tile([128, L], BF, tag="ka1", name=f"kt_a1_{f}")
                return {"a0": (q0, k0), "a1": (q1, k1)}

            for f in range(NQF):
                emit_qkproj("v", f, qt_v[f], kt_v[f])

            qka_f = {}
            for f in (0, 1):
                qka_f[f] = qka_tiles(f)
                for s in ("a0", "a1"):
                    emit_qkproj(s, f, *qka_f[f][s])
            drain(99)

            if apply_gb:
                nc.gpsimd.partition_broadcast(gb_bc[:, 0, :], gsb[:, :])
                nc.gpsimd.partition_broadcast(gb_bc[:, 1, :], bsb[:, :])

            # ---- attention ----
            def emit_attn_half(f, qh):
                qts = {"v": qt_v[f], "a0": qka_f[f]["a0"][0], "a1": qka_f[f]["a1"][0]}
                kts_t = {"v": kt_v[f], "a0": qka_f[f]["a0"][1], "a1": qka_f[f]["a1"][1]}
                kts = range(4) if qh == 0 else range(8)
                qsl = slice(qh * 512, (qh + 1) * 512)
                for hh in range(2):
                    h = 2 * f + hh
                    p0 = hh * 64
                    cps = ctx_ps.tile([DH + 1, 512], F32, tag="ctx", name=f"cps{f}{qh}{hh}")
                    pending = None  # software-pipeline ctx matmul one block back
                    for kt in kts:
                        sps = sc_ps.tile([128, 512], F32, tag="sc", name=f"sps{f}{qh}{hh}{kt}")
                        for j, s in enumerate(STREAMS):
                            nc.tensor.matmul(
                                sps[:, :],
                                kts_t[s][p0 : p0 + 64, kt * 128 : (kt + 1) * 128],
                                qts[s][p0 : p0 + 64, qsl],
                                start=(j == 0),
                                stop=(j == 2),
                            )
                        if pending is not None:
                            nc.tensor.matmul(
                                cps[:, :],
                                vnat[:, pending, h, :],
                                pending_attn[:, :],
                                start=(pending == 0),
                                stop=False,
                            )
                        if qh == 0 or kt >= 4:
                            slot = kt if qh == 0 else kt - 4
                            nc.vector.tensor_add(sps[:, :], sps[:, :], maskd_sb[:, slot, :])
                        attn_sb = attn_pool.tile([128, 512], BF, tag="attn", name=f"at{f}{qh}{hh}{kt}")
                        nc.scalar.activation(attn_sb[:, :], sps[:, :], AF.Exp, scale=SCALE)
                        pending, pending_attn = kt, attn_sb
                    nc.tensor.matmul(
                        cps[:, :],
                        vnat[:, pending, h, :],
                        pending_attn[:, :],
                        start=(pending == 0),
                        stop=True,
                    )
                    # normalize: denom -> SBUF, fast reciprocal, in-place bcast
                    den = small.tile([1, 512], F32, tag="den", bufs=1, name=f"den{f}{qh}{hh}")
                    inv_bc = small.tile([64, 512], F32, tag="invbc", bufs=2, name=f"invbc{f}{qh}{hh}")
                    nc.scalar.copy(den[:, :], cps[DH : DH + 1, :])
                    nc.vector.reciprocal_approx_fast(inv_bc[0:1, :], den[:, :])
                    nc.gpsimd.partition_broadcast(inv_bc[:, :], inv_bc[0:1, :])
                    nc.vector.tensor_mul(ctxt[p0 : p0 + 64, f, qsl], cps[0:DH, :], inv_bc[:, :])

            # ---- partial out-projection + pairwise ReduceScatter ----
            def emit_outproj_group(g):
                for li in range(2):
                    lt = 2 * g + li
                    ob = ob_pool.tile([128, D], BF, tag="ob", name=f"ob{g}{li}")
                    for dsl in range(2):
                        ps = qk_ps.tile([128, 512], F32, tag="qk", name=f"ops{g}{li}{dsl}")
                        for cf in range(NQF):
                            nc.tensor.matmul(
                                ps[:, :],
                                ctxt[:, cf, lt * 128 : (lt + 1) * 128],
                                wout_sb[:, cf, dsl * 512 : (dsl + 1) * 512],
                                start=(cf == 0),
                                stop=(cf == NQF - 1),
                            )
                        nc.vector.tensor_copy(ob[:, dsl * 512 : (dsl + 1) * 512], ps[:, :])
                    nc.sync.dma_start(out=partial_g[g][li * 128 : (li + 1) * 128, :], in_=ob[:, :])
                nc.gpsimd.collective_compute(
                    "ReduceScatter",
                    ALU.add,
                    replica_groups=PAIRS,
                    ins=[partial_g[g].opt()],
                    outs=[rs_g[g].opt()],
                )

            # schedule: proj f+2 between attn folds keeps the PE dense
            emit_attn_half(0, 0)
            emit_attn_half(0, 1)
            qka_f[2] = qka_tiles(2)
            for s in ("a0", "a1"):
                emit_qkproj(s, 2, *qka_f[2][s])
            emit_attn_half(1, 0)
            emit_attn_half(1, 1)
            qka_f[3] = qka_tiles(3)
            for s in ("a0", "a1"):
                emit_qkproj(s, 3, *qka_f[3][s])
            emit_attn_half(2, 0)
            emit_attn_half(2, 1)
            emit_attn_half(3, 0)
            emit_outproj_group(0)  # rows 0..255
            emit_outproj_group(1)  # rows 256..511
            emit_attn_half(3, 1)
            emit_outproj_group(2)  # rows 512..767
            emit_outproj_group(3)  # rows 768..1023

            # ---- residual + LayerNorm on own 4x128-row stripes ----
            for g in range(4):
                rs_rd = ln_pool.tile([128, D], BF, tag="rsrd", name=f"rsrd{g}")
                nc.sync.dma_start(out=rs_rd[:, :], in_=rs_g[g][:, :])
                x_sb = ln_pool.tile([128, D], F32, tag="x", name=f"lnx{g}")
                nc.vector.tensor_add(x_sb[:, :], rs_rd[:, :], ev_sb[:, g, :])
                stats = small.tile([128, 2, 6], F32, tag="stats", name=f"st{g}")
                nc.vector.bn_stats(out=stats[:, 0, :], in_=x_sb[:, 0:512])
                nc.vector.bn_stats(out=stats[:, 1, :], in_=x_sb[:, 512:1024])
                mv = small.tile([128, 2], F32, tag="mv", name=f"mv{g}")
                nc.vector.bn_aggr(out=mv[:, :], in_=stats[:, :, :])
                std = small.tile([128, 1], F32, tag="std", name=f"std{g}")
                nc.scalar.activation(std[:, :], mv[:, 1:2], AF.Sqrt, bias=eps_sb[:, :])
                rstd = small.tile([128, 1], F32, tag="rstd", name=f"rstd{g}")
                nc.vector.reciprocal(rstd[:, :], std[:, :])
                negmb = small.tile([128, 1], F32, tag="negmb", name=f"negmb{g}")
                nc.vector.scalar_tensor_tensor(
                    negmb[:, :],
                    mv[:, 0:1],
                    -1.0,
                    rstd[:, :],
                    op0=ALU.mult,
                    op1=ALU.mult,
                )
                nc.scalar.activation(
                    x_sb[:, :],
                    x_sb[:, :],
                    AF.Identity,
                    bias=negmb[:, :],
                    scale=rstd[:, :],
                )
                if apply_gb:
                    nc.gpsimd.tensor_mul(x_sb[:, :], x_sb[:, :], gb_bc[:, 0, :])
                    nc.gpsimd.tensor_add(x_sb[:, :], x_sb[:, :], gb_bc[:, 1, :])
                nc.sync.dma_start(out=out[g * 128 : (g + 1) * 128, :], in_=x_sb[:, :])

    nc.compile()
    return nc


def _get_nc(apply_gb: bool):
    key = ("nc", apply_gb)
    if key not in _NC_CACHE:
        _NC_CACHE[key] = build_nc(apply_gb)
    return _NC_CACHE[key]


def _pack_rows(a, dtype):
    """[R*128, X] row-major -> [128, R*X] with per-partition contiguous rows."""
    r = a.shape[0] // 128
    return np.ascontiguousarray(
        a.reshape(r, 128, -1).transpose(1, 0, 2).reshape(128, -1).astype(dtype)
    )


def prepare_in_maps(
    e_v, e_a0, e_a1, Wqv, bqv, Wkv, bkv, Wvv, bvv,
    Wqa0, bqa0, Wka0, bka0, Wqa1, bqa1, Wka1, bka1,
    Wout, bout, ln_gamma, ln_beta, attn_mask,
):
    f = np.asarray
    e = {"v": f(e_v), "a0": f(e_a0), "a1": f(e_a1)}
    attn_mask = f(attn_mask)
    Wout, bout, bvv = f(Wout), f(bout), f(bvv)
    c32 = lambda a: np.ascontiguousarray(np.asarray(a, dtype=np.float32))

    wq_full = {"v": f(Wqv), "a0": f(Wqa0), "a1": f(Wqa1)}
    wk_full = {"v": f(Wkv), "a0": f(Wka0), "a1": f(Wka1)}
    bq_full = {"v": f(bqv), "a0": f(bqa0), "a1": f(bqa1)}
    bk_full = {"v": f(bkv), "a0": f(bka0), "a1": f(bka1)}

    # bvv/bout are exactly absorbed by the post-reduce residual:
    # out = attn-ctx(Wvv-part) @ Wout + (bvv @ Wout + bout) + e_v
    resid_bias = (bvv.astype(np.float64) @ Wout.astype(np.float64) + bout).astype(np.float32)

    xts = {b: {s: _pack_rows(e[s][b].T, BF16) for s in STREAMS} for b in range(B)}
    maskds = {
        b: _pack_rows(attn_mask[b, 0, 0:512, 0:512].T * (1.0 / SCALE), BF16)
        for b in range(B)
    }

    def fold_pack(w, S):
        # [D, C-slice] -> [128, (f dt c)] with f=fold, dt=contraction tile
        ws = np.asarray(w[:, S], dtype=np.float32)  # [D, C]
        return np.ascontiguousarray(
            ws.reshape(NDT, 128, NQF, 128).transpose(1, 2, 0, 3).reshape(128, -1).astype(BF16)
        )

    in_maps = []
    for c in range(NCORES):
        b, hh = c // 2, c % 2
        S = slice(hh * C, (hh + 1) * C)
        m = {}
        for s in STREAMS:
            m[f"xt_{s}"] = xts[b][s]
            m[f"wq_{s}"] = fold_pack(wq_full[s], S)
            m[f"wk_{s}"] = fold_pack(wk_full[s], S)
            m[f"bq_{s}"] = c32(bq_full[s][S].reshape(NQF, 128).T)
            m[f"bk_{s}"] = c32(bk_full[s][S].reshape(NQF, 128).T)
        m["wv"] = _pack_rows(f(Wvv)[:, S].astype(np.float32), BF16)
        m["wout"] = _pack_rows(Wout[S, :].astype(np.float32), BF16)
        m["maskd"] = maskds[b]
        ev_rows = e["v"][b].reshape(4, 2, 128, D)[:, hh].astype(np.float32) + resid_bias
        m["ev_res"] = np.ascontiguousarray(ev_rows.transpose(1, 0, 2).reshape(128, -1).astype(BF16))
        m["gamma"] = np.ascontiguousarray(f(ln_gamma).astype(BF16)).reshape(1, D)
        m["beta"] = np.ascontiguousarray(f(ln_beta).astype(BF16)).reshape(1, D)
        in_maps.append(m)
    return in_maps


def kernel(**inputs):
    global LAST_RESULT
    in_maps = prepare_in_maps(**inputs)
    apply_gb = not (
        np.all(np.asarray(inputs["ln_gamma"]) == 1.0)
        and np.all(np.asarray(inputs["ln_beta"]) == 0.0)
    )

    nc = _get_nc(apply_gb)
    trace = bool(os.environ.get("KERNEL_TRACE"))
    res = run_bass_kernel_spmd(
        nc, in_maps, core_ids=list(range(NCORES)), trace=trace
    )
    LAST_RESULT = res

    out_full = np.empty((B, L, D), dtype=np.float32)
    for c in range(NCORES):
        b, hh = c // 2, c % 2
        o = res.results[c]["out"]  # [512, D]: 4 stripes of 128 rows
        for g in range(4):
            out_full[b, g * 256 + hh * 128 : g * 256 + hh * 128 + 128, :] = o[
                g * 128 : (g + 1) * 128
            ]
    return out_full


# revision 14
# speedup vs baseline: 1.6140x; 1.0067x over previous
"""DiffMHA (differential multi-head attention) block on 8 TRN2 NeuronCores.

Problem: B=4, L=1024, D=1024, H=16 heads (DH=64). Three input streams
(e_v, e_a0, e_a1); Q/K projections per stream, scores summed across
streams, causal-masked softmax, context from the v-stream values,
out-projection + residual + LayerNorm.

Sharding: (batch, head-half) -> 8 cores. Core c handles batch c//2 and
heads (c%2)*8 .. (c%2)*8+8. Each core computes its 8 heads' Q/K/V
projections (512 of 1024 channels), scores + softmax + context, and a
partial out-projection; four pairwise bf16 ReduceScatters (one per
256-row group) sum the two partial out-projections of a batch and
split rows; each core then applies residual + LayerNorm on its four
128-row stripes.

Performance notes (v3):
- PE p-state: the tensor engine only reaches 2.4 GHz after ~3 us of
  continuous execution; a warmup matmul burst starts the ramp while
  the first DMAs land, and the schedule keeps the PE queue dense
  (proj of fold f+2 emitted between attention folds).
- DMA staggering: only the V-projection critical path (wv, xt_v,
  wq_v, wk_v) is triggered at t=0 from the sync engine; every other
  load is triggered from the scalar engine interleaved between
  PSUM->SBUF copies, so the critical 3 MB is not bandwidth-starved.
- A zero-byte-ish dummy ReduceScatter at t=0 pre-pays the ~20 us
  collective channel setup so the first real RS is fast.
- Causal skip: q-half 0 computes only k-tiles 0..3; mask adds only on
  the 8 diagonal blocks per head; mask payload is one 512x512 block.
- Q/K biases ride the copy instructions (scalar activation for Q,
  vector tensor_scalar for K - splitting engines halves the copy
  backlog in front of the exps). V/out biases fold into the host-side
  residual (bvv @ Wout + bout), exact for all inputs.
- softmax in scores^T [k, q] layout: per-q sums via a ones-column in
  V; normalize = denom copy to SBUF + reciprocal_approx_fast +
  in-place partition broadcast (vector.reciprocal's 2.7us multipass
  and PSUM-input approx-reciprocal are both avoided).
- Epilogue: out-proj for rows 0..511 interleaves between the last
  fold's q-halves; PSUM->SBUF copies on gpsimd; 4 bf16 ReduceScatters
  overlap attention; LayerNorm skips gamma/beta when they are
  ones/zeros (runtime-selected graph variant).
"""

import os
import sys
import types

import ml_dtypes
import numpy as np

B, L, D, H = 4, 1024, 1024, 16
DH = D // H
HPC = H // 2  # heads per core
C = HPC * DH  # channels per core (512)
SCALE = float(1.0 / np.sqrt(DH))
EPS = 1e-12
NCORES = 8
BF16 = ml_dtypes.bfloat16


def _install_ntff_hook():
    """Recreate antenv.axon_hooks (absent in this image) so
    run_bass_kernel_spmd(trace=True) can capture NTFF profiles."""
    if "antenv.axon_hooks" in sys.modules:
        return
    try:
        from trn_agent_boot.trn_boot import _ntff_profile_via_ctypes

        hook = _ntff_profile_via_ctypes("/opt/axon/libaxon_pjrt.so")
    except Exception:
        hook = None
    mod = types.ModuleType("antenv.axon_hooks")
    mod.get_axon_ntff_profile_hook = lambda: hook
    mod.set_axon_ntff_profile_hook = lambda h: None
    sys.modules["antenv.axon_hooks"] = mod


_install_ntff_hook()

import concourse.bass as bass  # noqa: E402
import concourse.mybir as mybir  # noqa: E402
import concourse.tile as tile  # noqa: E402
from concourse import bacc  # noqa: E402
from concourse.bass_utils import run_bass_kernel_spmd  # noqa: E402

F32 = mybir.dt.float32
BF = mybir.dt.bfloat16
AF = mybir.ActivationFunctionType
ALU = mybir.AluOpType

_NC_CACHE = {}
LAST_RESULT = None

NQF = C // 128  # 4 channel folds per stream (2 heads each)
NLT = L // 128  # 8 l-tiles
NDT = D // 128  # 8 d-tiles (contraction)
NKT = L // 128  # 8 k-tiles
STREAMS = ("v", "a0", "a1")
PAIRS = [[0, 1], [2, 3], [4, 5], [6, 7]]
N_WARM = 12


def build_nc(apply_gb: bool):
    nc = bacc.Bacc("TRN2", target_bir_lowering=False, debug=False, num_devices=NCORES)

    # ---- DRAM parameters (per-core shards, host-packed to SBUF layout) ----
    xt = {s: nc.declare_dram_parameter(f"xt_{s}", [128, NDT * L], BF, isOutput=False) for s in STREAMS}
    wq = {s: nc.declare_dram_parameter(f"wq_{s}", [128, NQF * NDT * 128], BF, isOutput=False) for s in STREAMS}
    wk = {s: nc.declare_dram_parameter(f"wk_{s}", [128, NQF * NDT * 128], BF, isOutput=False) for s in STREAMS}
    wv = nc.declare_dram_parameter("wv", [128, NDT * C], BF, isOutput=False)
    wout = nc.declare_dram_parameter("wout", [128, NQF * D], BF, isOutput=False)
    bq = {s: nc.declare_dram_parameter(f"bq_{s}", [128, NQF], F32, isOutput=False) for s in STREAMS}
    bk = {s: nc.declare_dram_parameter(f"bk_{s}", [128, NQF], F32, isOutput=False) for s in STREAMS}
    maskd = nc.declare_dram_parameter("maskd", [128, 4 * 512], BF, isOutput=False)
    ev_res = nc.declare_dram_parameter("ev_res", [128, 4 * D], BF, isOutput=False)
    gamma = nc.declare_dram_parameter("gamma", [1, D], BF, isOutput=False)
    beta = nc.declare_dram_parameter("beta", [1, D], BF, isOutput=False)
    out = nc.declare_dram_parameter("out", [C, D], F32, isOutput=True)

    with tile.TileContext(nc) as tc:
        with (
            tc.tile_pool(name="persist", bufs=1) as persist,
            tc.tile_pool(name="qka", bufs=2) as qka,
            tc.tile_pool(name="small", bufs=4) as small,
            tc.tile_pool(name="attn", bufs=3) as attn_pool,
            tc.tile_pool(name="ln", bufs=2) as ln_pool,
            tc.tile_pool(name="ob", bufs=2) as ob_pool,
            tc.tile_pool(name="qk_ps", bufs=2, space="PSUM") as qk_ps,
            tc.tile_pool(name="sc_ps", bufs=3, space="PSUM") as sc_ps,
            tc.tile_pool(name="ctx_ps", bufs=3, space="PSUM") as ctx_ps,
            tc.tile_pool(name="dram", bufs=1, space="DRAM") as dram,
        ):
            # ---- persistent SBUF tensors ----
            warm = persist.tile([128, 512], BF, tag="warm")
            vnat = persist.tile([128, NLT, HPC, DH + 1], BF, tag="vnat")
            ctxt = persist.tile([128, NQF, L], BF, tag="ctxt")
            maskd_sb = persist.tile([128, 4, 512], BF, tag="maskd")
            wv_sb = persist.tile([128, NDT, C], BF, tag="wvsb")
            wout_sb = persist.tile([128, NQF, D], BF, tag="woutsb")
            ev_sb = persist.tile([128, 4, D], BF, tag="evsb")
            eps_sb = persist.tile([128, 1], F32, tag="eps")
            if apply_gb:
                gb_bc = persist.tile([128, 2, D], BF, tag="gbbc")
            xt_sb = {
                s: persist.tile([128, NDT, L], BF, tag=f"xt{s}", name=f"xt_sb_{s}")
                for s in STREAMS
            }
            wq_sb = {
                s: persist.tile([128, NQF, NDT, 128], BF, tag=f"wq{s}", name=f"wq_sb_{s}")
                for s in STREAMS
            }
            wk_sb = {
                s: persist.tile([128, NQF, NDT, 128], BF, tag=f"wk{s}", name=f"wk_sb_{s}")
                for s in STREAMS
            }
            bq_sb = {
                s: persist.tile([128, NQF], F32, tag=f"bq{s}", name=f"bq_sb_{s}")
                for s in STREAMS
            }
            bk_sb = {
                s: persist.tile([128, NQF], F32, tag=f"bk{s}", name=f"bk_sb_{s}")
                for s in STREAMS
            }
            # v-stream q/k tiles persist for all folds; a0/a1 ride a ring
            qt_v = [persist.tile([128, L], BF, tag=f"qtv{f}", name=f"qt_v{f}") for f in range(NQF)]
            kt_v = [persist.tile([128, L], BF, tag=f"ktv{f}", name=f"kt_v{f}") for f in range(NQF)]

            # DRAM scratch for the pairwise reduce-scatters
            partial_g = [dram.tile([256, D], BF, tag=f"part{g}", name=f"partial{g}") for g in range(4)]
            rs_g = [dram.tile([128, D], BF, tag=f"rs{g}", name=f"rsout{g}") for g in range(4)]
            wcc_in = dram.tile([2, 64], BF, tag="wccin")
            wcc_out = dram.tile([1, 64], BF, tag="wccout")

            # ---- t=0: critical-path loads only (sync engine), dt-chunked so
            #      the V projection can start after the first chunk ----
            for dt in range(NDT):
                nc.sync.dma_start(out=wv_sb[:, dt, :], in_=wv[:, dt * C : (dt + 1) * C])
                nc.sync.dma_start(
                    out=xt_sb["v"][:, dt, :], in_=xt["v"][:, dt * L : (dt + 1) * L])
            nc.sync.dma_start(out=wq_sb["v"][:, :, :, :], in_=wq["v"][:, :].rearrange("p (f dt c) -> p f dt c", f=NQF, dt=NDT))
            nc.sync.dma_start(out=wk_sb["v"][:, :, :, :], in_=wk["v"][:, :].rearrange("p (f dt c) -> p f dt c", f=NQF, dt=NDT))
            # dummy collective: pre-pays the CC channel setup (~20 us)
            nc.gpsimd.collective_compute(
                "ReduceScatter", ALU.add, replica_groups=PAIRS,
                ins=[wcc_in.opt()], outs=[wcc_out.opt()],
            )
            nc.vector.memset(warm[:, :], 1.0)
            nc.vector.memset(vnat[:, :, :, DH : DH + 1], 1.0)
            nc.vector.memset(eps_sb[:, :], EPS)

            # ---- deferred loads: triggered from the scalar engine between
            #      copies so they start only after the critical path is in ----
            deferred = []

            def defer(fn):
                deferred.append(fn)

            def drain(n=2):
                for _ in range(n):
                    if deferred:
                        deferred.pop(0)()

            for s in STREAMS:
                defer(lambda s=s: nc.scalar.dma_start(out=bq_sb[s][:, :], in_=bq[s][:, :]))
                defer(lambda s=s: nc.scalar.dma_start(out=bk_sb[s][:, :], in_=bk[s][:, :]))
            for s in ("a0", "a1"):
                defer(lambda s=s: nc.scalar.dma_start(
                    out=xt_sb[s][:, :, :], in_=xt[s][:, :].rearrange("p (dt l) -> p dt l", l=L)))

            def defer_fold_w(s, f):
                defer(lambda s=s, f=f: nc.scalar.dma_start(
                    out=wq_sb[s][:, f, :, :],
                    in_=wq[s][:, f * 1024 : (f + 1) * 1024].rearrange("p (dt c) -> p dt c", c=128)))
                defer(lambda s=s, f=f: nc.scalar.dma_start(
                    out=wk_sb[s][:, f, :, :],
                    in_=wk[s][:, f * 1024 : (f + 1) * 1024].rearrange("p (dt c) -> p dt c", c=128)))

            for s in ("a0", "a1"):
                defer_fold_w(s, 0)
            defer(lambda: nc.scalar.dma_start(
                out=maskd_sb[:, :, :], in_=maskd[:, :].rearrange("p (i q) -> p i q", q=512)))
            for s in ("a0", "a1"):
                defer_fold_w(s, 1)
            for f in (2, 3):
                for s in ("a0", "a1"):
                    defer_fold_w(s, f)
            defer(lambda: nc.scalar.dma_start(
                out=wout_sb[:, :, :], in_=wout[:, :].rearrange("p (cf d) -> p cf d", d=D)))
            defer(lambda: nc.scalar.dma_start(
                out=ev_sb[:, :, :], in_=ev_res[:, :].rearrange("p (i d) -> p i d", d=D)))
            if apply_gb:
                gsb = small.tile([1, D], BF, tag="gsb", bufs=1)
                bsb = small.tile([1, D], BF, tag="bsb", bufs=1)
                defer(lambda: nc.scalar.dma_start(out=gsb[:, :], in_=gamma[:, :]))
                defer(lambda: nc.scalar.dma_start(out=bsb[:, :], in_=beta[:, :]))

            # ---- PE warmup: ramp the p-state while DMAs land ----
            for i in range(N_WARM):
                wps = sc_ps.tile([128, 512], F32, tag="sc", name=f"warmps{i}")
                nc.tensor.matmul(wps[:, :], warm[:, 0:128], warm[:, :], start=True, stop=True)

            # ---- V projection, dt-outer across all 8 PSUM banks so compute
            #      starts as soon as the first dt chunk lands ----
            vps = (
                [sc_ps.tile([128, C], F32, tag="sc", name=f"vps{i}") for i in range(3)]
                + [ctx_ps.tile([128, C], F32, tag="ctx", name=f"vps{3+i}") for i in range(3)]
                + [qk_ps.tile([128, C], F32, tag="qk", name=f"vps{6+i}") for i in range(2)]
            )
            for dt in range(NDT):
                for lf in range(NLT):
                    nc.tensor.matmul(
                        vps[lf][:, :],
                        xt_sb["v"][:, dt, lf * 128 : (lf + 1) * 128],
                        wv_sb[:, dt, :],
                        start=(dt == 0),
                        stop=(dt == NDT - 1),
                    )
            for lf in range(NLT):
                nc.scalar.copy(vnat[:, lf, :, 0:DH], vps[lf][:, :])
                drain(2)

            # ---- Q/K projections: Q copy on scalar, K copy on vector ----
            def emit_qkproj(s, f, qdst, kdst):
                for which, w_sb, b_sb, dst in (
                    ("q", wq_sb[s], bq_sb[s], qdst),
                    ("k", wk_sb[s], bk_sb[s], kdst),
                ):
                    for lh in range(2):
                        ps = qk_ps.tile([128, 512], F32, tag="qk", name=f"pj{s}{f}{lh}")
                        for dt in range(NDT):
                            nc.tensor.matmul(
                                ps[:, :],
                                w_sb[:, f, dt, :],
                                xt_sb[s][:, dt, lh * 512 : (lh + 1) * 512],
                                start=(dt == 0),
                                stop=(dt == NDT - 1),
                            )
                        dsl = dst[:, lh * 512 : (lh + 1) * 512]
                        if which == "q":
                            nc.scalar.activation(dsl, ps[:, :], AF.Identity, bias=b_sb[:, f : f + 1])
                            drain(2)
                        else:
                            nc.vector.tensor_scalar(
                                dsl, ps[:, :], b_sb[:, f : f + 1], None, ALU.add
                            )

            def qka_tiles(f):
                q0 = qka.tile([128, L], BF, tag="qa0", name=f"qt_a0_{f}")
                k0 = qka.tile([128, L], BF, tag="ka0", name=f"kt_a0_{f}")
                q1 = qka.tile([128, L], BF, tag="qa1", name=f"qt_a1_{f}")
                k1 = qka.tile([128, L], BF, tag="ka1", name=f"kt_a1_{f}")
                return {"a0": (q0, k0), "a1": (q1, k1)}

            for f in range(NQF):
                emit_qkproj("v", f, qt_v[f], kt_v[f])

            qka_f = {}
            for f in (0, 1):
                qka_f[f] = qka_tiles(f)
                for s in ("a0", "a1"):
                    emit_qkproj(s, f, *qka_f[f][s])
            drain(99)

            if apply_gb:
                nc.gpsimd.partition_broadcast(gb_bc[:, 0, :], gsb[:, :])
                nc.gpsimd.partition_broadcast(gb_bc[:, 1, :], bsb[:, :])

            # ---- attention ----
            def emit_attn_half(f, qh):
                qts = {"v": qt_v[f], "a0": qka_f[f]["a0"][0], "a1": qka_f[f]["a1"][0]}
                kts_t = {"v": kt_v[f], "a0": qka_f[f]["a0"][1], "a1": qka_f[f]["a1"][1]}
                kts = range(4) if qh == 0 else range(8)
                qsl = slice(qh * 512, (qh + 1) * 512)
                for hh in range(2):
                    h = 2 * f + hh
                    p0 = hh * 64
                    cps = ctx_ps.tile([DH + 1, 512], F32, tag="ctx", name=f"cps{f}{qh}{hh}")
                    pending = None  # software-pipeline ctx matmul one block back
                    for kt in kts:
                        sps = sc_ps.tile([128, 512], F32, tag="sc", name=f"sps{f}{qh}{hh}{kt}")
                        for j, s in enumerate(STREAMS):
                            nc.tensor.matmul(
                                sps[:, :],
                                kts_t[s][p0 : p0 + 64, kt * 128 : (kt + 1) * 128],
                                qts[s][p0 : p0 + 64, qsl],
                                start=(j == 0),
                                stop=(j == 2),
                            )
                        if pending is not None:
                            nc.tensor.matmul(
                                cps[:, :],
                                vnat[:, pending, h, :],
                                pending_attn[:, :],
                                start=(pending == 0),
                                stop=False,
                            )
                        if qh == 0 or kt >= 4:
                            slot = kt if qh == 0 else kt - 4
                            nc.vector.tensor_add(sps[:, :], sps[:, :], maskd_sb[:, slot, :])
                        attn_sb = attn_pool.tile([128, 512], BF, tag="attn", name=f"at{f}{qh}{hh}{kt}")
                        nc.scalar.activation(attn_sb[:, :], sps[:, :], AF.Exp, scale=SCALE)
                        pending, pending_attn = kt, attn_sb
                    nc.tensor.matmul(
                        cps[:, :],
                        vnat[:, pending, h, :],
                        pending_attn[:, :],
                        start=(pending == 0),
                        stop=True,
                    )
                    # normalize: denom -> SBUF, fast reciprocal, in-place bcast
                    den = small.tile([1, 512], F32, tag="den", bufs=1, name=f"den{f}{qh}{hh}")
                    inv_bc = small.tile([64, 512], F32, tag="invbc", bufs=2, name=f"invbc{f}{qh}{hh}")
                    nc.vector.tensor_copy(den[:, :], cps[DH : DH + 1, :])
                    nc.vector.reciprocal_approx_fast(inv_bc[0:1, :], den[:, :])
                    nc.gpsimd.partition_broadcast(inv_bc[:, :], inv_bc[0:1, :])
                    nc.vector.tensor_mul(ctxt[p0 : p0 + 64, f, qsl], cps[0:DH, :], inv_bc[:, :])

            # ---- partial out-projection + pairwise ReduceScatter ----
            def emit_outproj_group(g):
                for li in range(2):
                    lt = 2 * g + li
                    ob = ob_pool.tile([128, D], BF, tag="ob", name=f"ob{g}{li}")
                    for dsl in range(2):
                        ps = qk_ps.tile([128, 512], F32, tag="qk", name=f"ops{g}{li}{dsl}")
                        for cf in range(NQF):
                            nc.tensor.matmul(
                                ps[:, :],
                                ctxt[:, cf, lt * 128 : (lt + 1) * 128],
                                wout_sb[:, cf, dsl * 512 : (dsl + 1) * 512],
                                start=(cf == 0),
                                stop=(cf == NQF - 1),
                            )
                        nc.vector.tensor_copy(ob[:, dsl * 512 : (dsl + 1) * 512], ps[:, :])
                    nc.sync.dma_start(out=partial_g[g][li * 128 : (li + 1) * 128, :], in_=ob[:, :])
                nc.gpsimd.collective_compute(
                    "ReduceScatter",
                    ALU.add,
                    replica_groups=PAIRS,
                    ins=[partial_g[g].opt()],
                    outs=[rs_g[g].opt()],
                )

            # schedule: proj f+2 between attn folds keeps the PE dense
            emit_attn_half(0, 0)
            emit_attn_half(0, 1)
            qka_f[2] = qka_tiles(2)
            for s in ("a0", "a1"):
                emit_qkproj(s, 2, *qka_f[2][s])
            emit_attn_half(1, 0)
            emit_attn_half(1, 1)
            qka_f[3] = qka_tiles(3)
            for s in ("a0", "a1"):
                emit_qkproj(s, 3, *qka_f[3][s])
            emit_attn_half(2, 0)
            emit_attn_half(2, 1)
            emit_attn_half(3, 1)
            emit_outproj_group(2)  # rows 512..767
            emit_outproj_group(3)  # rows 768..1023
            emit_attn_half(3, 0)
            emit_outproj_group(0)  # rows 0..255
            emit_outproj_group(1)  # rows 256..511

            # ---- residual + LayerNorm on own 4x128-row stripes ----
            for g in (2, 3, 0, 1):
                rs_rd = ln_pool.tile([128, D], BF, tag="rsrd", name=f"rsrd{g}")
                nc.sync.dma_start(out=rs_rd[:, :], in_=rs_g[g][:, :])
                x_sb = ln_pool.tile([128, D], F32, tag="x", name=f"lnx{g}")
                nc.gpsimd.tensor_add(x_sb[:, :], rs_rd[:, :], ev_sb[:, g, :])
                stats = small.tile([128, 2, 6], F32, tag="stats", name=f"st{g}")
                nc.vector.bn_stats(out=stats[:, 0, :], in_=x_sb[:, 0:512])
                nc.vector.bn_stats(out=stats[:, 1, :], in_=x_sb[:, 512:1024])
                mv = small.tile([128, 2], F32, tag="mv", name=f"mv{g}")
                nc.vector.bn_aggr(out=mv[:, :], in_=stats[:, :, :])
                std = small.tile([128, 1], F32, tag="std", name=f"std{g}")
                nc.scalar.activation(std[:, :], mv[:, 1:2], AF.Sqrt, bias=eps_sb[:, :])
                rstd = small.tile([128, 1], F32, tag="rstd", name=f"rstd{g}")
                nc.vector.reciprocal(rstd[:, :], std[:, :])
                negmb = small.tile([128, 1], F32, tag="negmb", name=f"negmb{g}")
                nc.vector.scalar_tensor_tensor(
                    negmb[:, :],
                    mv[:, 0:1],
                    -1.0,
                    rstd[:, :],
                    op0=ALU.mult,
                    op1=ALU.mult,
                )
                nc.scalar.activation(
                    x_sb[:, :],
                    x_sb[:, :],
                    AF.Identity,
                    bias=negmb[:, :],
                    scale=rstd[:, :],
                )
                if apply_gb:
                    nc.gpsimd.tensor_mul(x_sb[:, :], x_sb[:, :], gb_bc[:, 0, :])
                    nc.gpsimd.tensor_add(x_sb[:, :], x_sb[:, :], gb_bc[:, 1, :])
                nc.sync.dma_start(out=out[g * 128 : (g + 1) * 128, :], in_=x_sb[:, :])

    nc.compile()
    return nc


def _get_nc(apply_gb: bool):
    key = ("nc", apply_gb)
    if key not in _NC_CACHE:
        _NC_CACHE[key] = build_nc(apply_gb)
    return _NC_CACHE[key]


def _pack_rows(a, dtype):
    """[R*128, X] row-major -> [128, R*X] with per-partition contiguous rows."""
    r = a.shape[0] // 128
    return np.ascontiguousarray(
        a.reshape(r, 128, -1).transpose(1, 0, 2).reshape(128, -1).astype(dtype)
    )


def prepare_in_maps(
    e_v, e_a0, e_a1, Wqv, bqv, Wkv, bkv, Wvv, bvv,
    Wqa0, bqa0, Wka0, bka0, Wqa1, bqa1, Wka1, bka1,
    Wout, bout, ln_gamma, ln_beta, attn_mask,
):
    f = np.asarray
    e = {"v": f(e_v), "a0": f(e_a0), "a1": f(e_a1)}
    attn_mask = f(attn_mask)
    Wout, bout, bvv = f(Wout), f(bout), f(bvv)
    c32 = lambda a: np.ascontiguousarray(np.asarray(a, dtype=np.float32))

    wq_full = {"v": f(Wqv), "a0": f(Wqa0), "a1": f(Wqa1)}
    wk_full = {"v": f(Wkv), "a0": f(Wka0), "a1": f(Wka1)}
    bq_full = {"v": f(bqv), "a0": f(bqa0), "a1": f(bqa1)}
    bk_full = {"v": f(bkv), "a0": f(bka0), "a1": f(bka1)}

    # bvv/bout are exactly absorbed by the post-reduce residual:
    # out = attn-ctx(Wvv-part) @ Wout + (bvv @ Wout + bout) + e_v
    resid_bias = (bvv.astype(np.float64) @ Wout.astype(np.float64) + bout).astype(np.float32)

    xts = {b: {s: _pack_rows(e[s][b].T, BF16) for s in STREAMS} for b in range(B)}
    maskds = {
        b: _pack_rows(attn_mask[b, 0, 0:512, 0:512].T * (1.0 / SCALE), BF16)
        for b in range(B)
    }

    def fold_pack(w, S):
        # [D, C-slice] -> [128, (f dt c)] with f=fold, dt=contraction tile
        ws = np.asarray(w[:, S], dtype=np.float32)  # [D, C]
        return np.ascontiguousarray(
            ws.reshape(NDT, 128, NQF, 128).transpose(1, 2, 0, 3).reshape(128, -1).astype(BF16)
        )

    in_maps = []
    for c in range(NCORES):
        b, hh = c // 2, c % 2
        S = slice(hh * C, (hh + 1) * C)
        m = {}
        for s in STREAMS:
            m[f"xt_{s}"] = xts[b][s]
            m[f"wq_{s}"] = fold_pack(wq_full[s], S)
            m[f"wk_{s}"] = fold_pack(wk_full[s], S)
            m[f"bq_{s}"] = c32(bq_full[s][S].reshape(NQF, 128).T)
            m[f"bk_{s}"] = c32(bk_full[s][S].reshape(NQF, 128).T)
        m["wv"] = _pack_rows(f(Wvv)[:, S].astype(np.float32), BF16)
        m["wout"] = _pack_rows(Wout[S, :].astype(np.float32), BF16)
        m["maskd"] = maskds[b]
        ev_rows = e["v"][b].reshape(4, 2, 128, D)[:, hh].astype(np.float32) + resid_bias
        m["ev_res"] = np.ascontiguousarray(ev_rows.transpose(1, 0, 2).reshape(128, -1).astype(BF16))
        m["gamma"] = np.ascontiguousarray(f(ln_gamma).astype(BF16)).reshape(1, D)
        m["beta"] = np.ascontiguousarray(f(ln_beta).astype(BF16)).reshape(1, D)
        in_maps.append(m)
    return in_maps


def kernel(**inputs):
    global LAST_RESULT
    in_maps = prepare_in_maps(**inputs)
    apply_gb = not (
        np.all(np.asarray(inputs["ln_gamma"]) == 1.0)
        and np.all(np.asarray(inputs["ln_beta"]) == 0.0)
    )

    nc = _get_nc(apply_gb)
    trace = bool(os.environ.get("KERNEL_TRACE"))
    res = run_bass_kernel_spmd(
        nc, in_maps, core_ids=list(range(NCORES)), trace=trace
    )
    LAST_RESULT = res

    out_full = np.empty((B, L, D), dtype=np.float32)
    for c in range(NCORES):
        b, hh = c // 2, c % 2
        o = res.results[c]["out"]  # [512, D]: 4 stripes of 128 rows
        for g in range(4):
            out_full[b, g * 256 + hh * 128 : g * 256 + hh * 128 + 128, :] = o[
                g * 128 : (g + 1) * 128
            ]
    return out_full
